# revision 5
# baseline (speedup 1.0000x reference)
"""Triangular pairwise channel product on 8 Trainium2 NeuronCores.

out[b,h,w,k] = x[b,h,w,i_k] * x[b,h,w,j_k]  for the C*(C-1)/2 pairs
(i<j) in row-major (np.triu_indices) order.

Sharding: pure data parallel over batch — core c takes x[2c:2c+2].
Per core the 2*64*64 = 8192 spatial positions map to 128 SBUF
partitions (b_loc*64+h) x 64 groups (w).  Block i of the output (pairs
(i, i+1..63)) is one fp32 tensor_tensor multiply per group-chunk whose
first operand is x[..., i] broadcast via a step-0 access pattern.

The kernel is HBM-store bound: 66 MB of output per core at ~420 GB/s
observed DMA ≈ 160 us of unavoidable store time.  Design, driven by
traces:

* Work split: blocks i < I0 on DVE (1 cyc/elem + ~145 cyc/instr), the
  small tail blocks on GPSIMD (~1.7 ns/elem + ~190 ns/instr, otherwise
  idle).  DVE alone (~190 us) cannot keep the store stream fed.
* fp32 TT on DVE reads its second operand through the SBUF port pair
  GPSIMD uses — running both engines naively serializes them to
  ~2.3 cyc/elem.  Fix: DVE's broadcast operand lives in PSUM (separate
  DVE read port).  The PE — otherwise dead — stages each x chunk
  SBUF->PSUM via an identity matmul (I.T @ x), keeping the staging off
  every contended sequencer.
* Stores are monolithic [P, Gi, K] rows (contiguous per-partition DRAM
  runs, 128 descriptors; channel-split half stores cost ~9 us of
  sequencer descriptor-gen each and serialized the rings).  Rings
  alternate per iteration.
* bufs=3 on the output pool: engine completion semaphores update ~5 us
  late (event-accel spacing), and with bufs=2 that lag plus the
  store->compute->store chain left the DMA idle ~30% of the time.  A
  third buffer makes the store stream the only binding constraint.
"""

import numpy as np

import concourse.bacc as bacc
import concourse.bass as bass
import concourse.mybir as mybir
import concourse.tile as tile
from concourse.bass_utils import run_bass_kernel_spmd

B, H, W, C = 16, 64, 64, 64
K = C * (C - 1) // 2  # 2016
N_CORES = 8
BP = B // N_CORES  # batch rows per core
P = BP * H         # 128 SBUF partitions
G_TOTAL = W        # position groups per partition
G_ITERS = [2, 6, 8, 8, 8, 8, 8, 8, 5, 3]
assert sum(G_ITERS) == W
G0 = G_ITERS[0]
GMAX = max(G_ITERS)
# Blocks i >= I0 (widths 63-I0 .. 1) run on GPSIMD, the rest on DVE.
I0 = 30
FP = mybir.dt.float32
MM_CHUNK = 512 // C  # groups per identity-matmul (moving free dim <= 512)

_row = [0]
for _i in range(C):
    _row.append(_row[-1] + C - 1 - _i)

_nc_cache = None


def build_bass() -> bass.Bass:
    # Bacc (not plain Bass): its compile() pipeline runs
    # generate_event_semaphores, which splits multi-wait instructions to
    # satisfy the TRN2 1-wait-per-instruction codegen limit.
    nc = bacc.Bacc(
        "TRN2",
        target_bir_lowering=False,
        debug=False,
        num_devices=N_CORES,
    )
    x = nc.dram_tensor("x", [P, G_TOTAL, C], FP, kind="ExternalInput")
    ident = nc.dram_tensor("ident", [P, P], FP, kind="ExternalInput")
    y = nc.dram_tensor("y", [P, G_TOTAL, K], FP, kind="ExternalOutput")

    with tile.TileContext(nc) as tc:
        with (
            tc.tile_pool(name="xin", bufs=1) as xpool,
            tc.tile_pool(name="out", bufs=3) as opool,
            tc.tile_pool(name="xps", bufs=2, space="PSUM") as ppool,
        ):
            # One x tile; iteration 0's chunk loads on the SP ring, the
            # rest (+ identity) on the ACT ring so the first compute
            # starts as early as possible.
            xt = xpool.tile([P, G_TOTAL, C], FP, tag="x")
            nc.sync.dma_start(out=xt[:, 0:G0, :], in_=x[:, 0:G0, :])
            idt = xpool.tile([P, P], FP, tag="id")
            nc.sync.dma_start(out=idt[:], in_=ident[:, :])
            nc.scalar.dma_start(out=xt[:, G0:, :], in_=x[:, G0:, :])

            g_off = 0
            for it, Gi in enumerate(G_ITERS):
                xg = xt[:, g_off : g_off + Gi, :]

                # PE stages the chunk into PSUM (xp = I.T @ xg) for DVE's
                # broadcast operands, keeping DVE off the GPSIMD-shared
                # SBUF port and the staging off the DMA sequencers.
                xp = ppool.tile([P, GMAX, C], FP, tag="xp")
                for c0 in range(0, Gi, MM_CHUNK):
                    c1 = min(c0 + MM_CHUNK, Gi)
                    nc.tensor.matmul(
                        out=xp[:, c0:c1, :], lhsT=idt[:], rhs=xg[:, c0:c1, :]
                    )

                ot = opool.tile([P, Gi, K], FP, tag="ot")

                # GPSIMD tail blocks first so its queue starts immediately
                # (all-SBUF operands; DVE never touches the shared pair).
                for i in range(I0, C - 1):
                    w = C - 1 - i
                    a = xg[:, :, i : i + 1].broadcast_to([P, Gi, w])
                    b = xg[:, :, i + 1 : C]
                    nc.gpsimd.tensor_mul(ot[:, :, _row[i] : _row[i] + w], a, b)

                for i in range(0, I0):
                    w = C - 1 - i
                    a = xp[:, 0:Gi, i : i + 1].broadcast_to([P, Gi, w])
                    b = xg[:, :, i + 1 : C]
                    nc.vector.tensor_mul(ot[:, :, _row[i] : _row[i] + w], a, b)

                # Full 2016-channel rows -> contiguous per-partition DRAM
                # runs; alternate HWDGE rings.
                ring = nc.sync if it % 2 == 0 else nc.scalar
                ring.dma_start(out=y[:, g_off : g_off + Gi, :], in_=ot[:])
                g_off += Gi

    nc.finalize()
    return nc


def make_in_maps(x: np.ndarray) -> list[dict[str, np.ndarray]]:
    x = np.ascontiguousarray(x, dtype=np.float32)
    eye = np.eye(P, dtype=np.float32)
    return [
        {"x": x[c * BP : (c + 1) * BP].reshape(P, G_TOTAL, C), "ident": eye}
        for c in range(N_CORES)
    ]


def kernel(**inputs: np.ndarray) -> np.ndarray:
    global _nc_cache
    if _nc_cache is None:
        _nc_cache = build_bass()
    res = run_bass_kernel_spmd(
        _nc_cache, make_in_maps(inputs["inputs"]), list(range(N_CORES))
    ).results
    return np.concatenate(
        [res[c]["y"].reshape(BP, H, W, K) for c in range(N_CORES)], axis=0
    )


# revision 6
# speedup vs baseline: 1.4950x; 1.4950x over previous
"""Triangular pairwise channel product on 8 Trainium2 NeuronCores.

out[b,h,w,k] = x[b,h,w,i_k] * x[b,h,w,j_k]  for the C*(C-1)/2 pairs
(i<j) in row-major (np.triu_indices) order.

Sharding: pure data parallel over batch — core c takes x[2c:2c+2].
Per core the 2*64*64 = 8192 spatial positions map to 128 SBUF
partitions (b_loc*64+h) x 64 groups (w).  Block i of the output (pairs
(i, i+1..63)) is one tensor_tensor multiply per group-chunk whose first
operand is x[..., i] broadcast via a step-0 access pattern.

Tracing showed the fp32 kernel is bound by DEVICE HBM bandwidth: all 8
cores stream stores simultaneously and sustain only ~350 GB/s each
(~2.8 TB/s device), so 528 MB of fp32 output floors at ~196 us no
matter the schedule.  The rel-err budget is 2e-2; bf16 products carry
~2e-3.  So x is cast to bf16 on the host, all products are computed
from bf16 inputs and stored as bf16 (264 MB device-wide, ~100 us
floor), and the host upcasts the result.

With stores halved, compute paces the kernel; design from fp32 traces:

* Work split: blocks i < I0 on DVE (1 cyc/elem @0.96 + ~145 cyc/instr),
  tail blocks on GPSIMD (~1.7 ns/elem + ~190 ns/instr, otherwise idle).
* fp32/bf16 TT on DVE reads its second operand through the SBUF port
  pair GPSIMD uses — running both engines naively serializes them to
  ~2.3 cyc/elem.  Fix: DVE's broadcast operand lives in PSUM (separate
  DVE read port).  The PE — otherwise dead — stages each x chunk
  SBUF->PSUM via an identity matmul (bf16 in, exact fp32 out), keeping
  the staging off every contended sequencer.
* Stores are monolithic [P, Gi, K] rows (contiguous per-partition DRAM
  runs, 128 descriptors), alternating HWDGE rings; bufs=3 so the ~5 us
  engine-semaphore lag never gates the store stream.
"""

import numpy as np

import concourse.bacc as bacc
import concourse.bass as bass
import concourse.mybir as mybir
import concourse.tile as tile
from concourse.bass_utils import run_bass_kernel_spmd

B, H, W, C = 16, 64, 64, 64
K = C * (C - 1) // 2  # 2016
N_CORES = 8
BP = B // N_CORES  # batch rows per core
P = BP * H         # 128 SBUF partitions
G_TOTAL = W        # position groups per partition
G_ITERS = [2, 6, 16, 16, 16, 6, 2]
assert sum(G_ITERS) == W
G0 = G_ITERS[0]
GMAX = max(G_ITERS)
# Blocks i >= I0 (widths 63-I0 .. 1) run on GPSIMD, the rest on DVE.
I0 = 26
FP = mybir.dt.float32
BF = mybir.dt.bfloat16
NP_BF = mybir.dt.np(BF)
MM_CHUNK = 512 // C  # groups per identity-matmul (moving free dim <= 512)

_row = [0]
for _i in range(C):
    _row.append(_row[-1] + C - 1 - _i)

_nc_cache = None


def build_bass() -> bass.Bass:
    # Bacc (not plain Bass): its compile() pipeline runs
    # generate_event_semaphores, which splits multi-wait instructions to
    # satisfy the TRN2 1-wait-per-instruction codegen limit.
    nc = bacc.Bacc(
        "TRN2",
        target_bir_lowering=False,
        debug=False,
        num_devices=N_CORES,
    )
    x = nc.dram_tensor("x", [P, G_TOTAL, C], BF, kind="ExternalInput")
    ident = nc.dram_tensor("ident", [P, P], BF, kind="ExternalInput")
    y = nc.dram_tensor("y", [P, G_TOTAL, K], BF, kind="ExternalOutput")

    with tile.TileContext(nc) as tc:
        with (
            tc.tile_pool(name="xin", bufs=1) as xpool,
            tc.tile_pool(name="out", bufs=3) as opool,
            tc.tile_pool(name="xps", bufs=2, space="PSUM") as ppool,
        ):
            # One x tile; iteration 0's chunk loads on the SP ring, the
            # rest (+ identity) on the ACT ring so the first compute
            # starts as early as possible.
            xt = xpool.tile([P, G_TOTAL, C], BF, tag="x")
            nc.sync.dma_start(out=xt[:, 0:G0, :], in_=x[:, 0:G0, :])
            idt = xpool.tile([P, P], BF, tag="id")
            nc.sync.dma_start(out=idt[:], in_=ident[:, :])
            nc.scalar.dma_start(out=xt[:, G0:, :], in_=x[:, G0:, :])

            g_off = 0
            for it, Gi in enumerate(G_ITERS):
                xg = xt[:, g_off : g_off + Gi, :]

                # PE stages the chunk into PSUM (xp = I.T @ xg, exact) for
                # DVE's broadcast operands, keeping DVE off the
                # GPSIMD-shared SBUF port.
                xp = ppool.tile([P, GMAX, C], FP, tag="xp")
                for c0 in range(0, Gi, MM_CHUNK):
                    c1 = min(c0 + MM_CHUNK, Gi)
                    nc.tensor.matmul(
                        out=xp[:, c0:c1, :], lhsT=idt[:], rhs=xg[:, c0:c1, :]
                    )

                ot = opool.tile([P, Gi, K], BF, tag="ot")

                # GPSIMD tail blocks first so its queue starts immediately
                # (all-SBUF operands; DVE never touches the shared pair).
                for i in range(I0, C - 1):
                    w = C - 1 - i
                    a = xg[:, :, i : i + 1].broadcast_to([P, Gi, w])
                    b = xg[:, :, i + 1 : C]
                    nc.gpsimd.tensor_mul(ot[:, :, _row[i] : _row[i] + w], a, b)

                for i in range(0, I0):
                    w = C - 1 - i
                    a = xp[:, 0:Gi, i : i + 1].broadcast_to([P, Gi, w])
                    b = xg[:, :, i + 1 : C]
                    nc.vector.tensor_mul(ot[:, :, _row[i] : _row[i] + w], a, b)

                # Full 2016-channel rows -> contiguous per-partition DRAM
                # runs; alternate HWDGE rings.
                ring = nc.sync if it % 2 == 0 else nc.scalar
                ring.dma_start(out=y[:, g_off : g_off + Gi, :], in_=ot[:])
                g_off += Gi

    nc.finalize()
    return nc


def make_in_maps(x: np.ndarray) -> list[dict[str, np.ndarray]]:
    x = np.ascontiguousarray(x, dtype=np.float32).astype(NP_BF)
    eye = np.eye(P, dtype=np.float32).astype(NP_BF)
    return [
        {"x": x[c * BP : (c + 1) * BP].reshape(P, G_TOTAL, C), "ident": eye}
        for c in range(N_CORES)
    ]


def kernel(**inputs: np.ndarray) -> np.ndarray:
    global _nc_cache
    if _nc_cache is None:
        _nc_cache = build_bass()
    res = run_bass_kernel_spmd(
        _nc_cache, make_in_maps(inputs["inputs"]), list(range(N_CORES))
    ).results
    return np.concatenate(
        [
            res[c]["y"].astype(np.float32).reshape(BP, H, W, K)
            for c in range(N_CORES)
        ],
        axis=0,
    )


# revision 7
# speedup vs baseline: 1.5699x; 1.0501x over previous
"""Triangular pairwise channel product on 8 Trainium2 NeuronCores.

out[b,h,w,k] = x[b,h,w,i_k] * x[b,h,w,j_k]  for the C*(C-1)/2 pairs
(i<j) in row-major (np.triu_indices) order.

Sharding: pure data parallel over batch — core c takes x[2c:2c+2].
Per core the 2*64*64 = 8192 spatial positions map to 128 SBUF
partitions (b_loc*64+h) x 64 groups (w).  Block i of the output (pairs
(i, i+1..63)) is one tensor_tensor multiply per group-chunk whose first
operand is x[..., i] broadcast via a step-0 access pattern.

Tracing showed the fp32 kernel is bound by DEVICE HBM bandwidth: all 8
cores stream stores simultaneously and sustain only ~350 GB/s each
(~2.8 TB/s device), so 528 MB of fp32 output floors at ~196 us no
matter the schedule.  The rel-err budget is 2e-2; bf16 products carry
~3e-3.  So x is cast to bf16 on the host, all products are computed
from bf16 inputs and stored as bf16 (264 MB device-wide, ~100 us
floor), and the host upcasts the result.

Compute paces the kernel now; design points from traces:

* Work split: blocks i < I0 on DVE (1 cyc/elem @0.96 + ~145 cyc/instr),
  tail blocks on GPSIMD (~1.9 ns/elem + ~190 ns/instr, otherwise idle).
* TT on DVE reads its second operand through the SBUF port pair GPSIMD
  uses — running both engines naively serializes them to ~2.3 cyc/elem.
  Fix: DVE's broadcast operand lives in PSUM (separate DVE read port).
  The PE — otherwise dead — stages each x chunk SBUF->PSUM via an
  identity matmul (bf16 in, exact fp32 out).
* Each engine owns a separate DRAM output tensor (DVE rows 0..R0,
  GPSIMD rows R0..K; the host concatenates channels during unshard).
  A shared tile made every store wait on BOTH engines' completion
  semaphores, whose sparse increments rounded the wait up by as much as
  20 instructions (~17 us of dead DMA time per store).  Separate
  tensors keep every store single-engine, contiguous per partition in
  DRAM (128 descriptors), and on a dedicated HWDGE ring per engine.
"""

import numpy as np

import concourse.bacc as bacc
import concourse.bass as bass
import concourse.mybir as mybir
import concourse.tile as tile
from concourse.bass_utils import run_bass_kernel_spmd

B, H, W, C = 16, 64, 64, 64
K = C * (C - 1) // 2  # 2016
N_CORES = 8
BP = B // N_CORES  # batch rows per core
P = BP * H         # 128 SBUF partitions
G_TOTAL = W        # position groups per partition
G_ITERS = [2, 6, 16, 16, 16, 6, 2]
assert sum(G_ITERS) == W
G0 = G_ITERS[0]
GMAX = max(G_ITERS)
# Blocks i >= I0 (widths 63-I0 .. 1) run on GPSIMD, the rest on DVE.
I0 = 26
FP = mybir.dt.float32
BF = mybir.dt.bfloat16
NP_BF = mybir.dt.np(BF)
MM_CHUNK = 512 // C  # groups per identity-matmul (moving free dim <= 512)

_row = [0]
for _i in range(C):
    _row.append(_row[-1] + C - 1 - _i)
R0 = _row[I0]  # first GPSIMD-owned output channel

_nc_cache = None


def build_bass() -> bass.Bass:
    # Bacc (not plain Bass): its compile() pipeline runs
    # generate_event_semaphores, which splits multi-wait instructions to
    # satisfy the TRN2 1-wait-per-instruction codegen limit.
    nc = bacc.Bacc(
        "TRN2",
        target_bir_lowering=False,
        debug=False,
        num_devices=N_CORES,
    )
    x = nc.dram_tensor("x", [P, G_TOTAL, C], BF, kind="ExternalInput")
    ident = nc.dram_tensor("ident", [P, P], BF, kind="ExternalInput")
    yv = nc.dram_tensor("yv", [P, G_TOTAL, R0], BF, kind="ExternalOutput")
    yg = nc.dram_tensor("yg", [P, G_TOTAL, K - R0], BF, kind="ExternalOutput")

    with tile.TileContext(nc) as tc:
        with (
            tc.tile_pool(name="xin", bufs=1) as xpool,
            tc.tile_pool(name="outv", bufs=3) as vpool,
            tc.tile_pool(name="outg", bufs=3) as gpool,
            tc.tile_pool(name="xps", bufs=2, space="PSUM") as ppool,
        ):
            # One x tile; iteration 0's chunk loads on the SP ring, the
            # rest (+ identity) on the ACT ring so the first compute
            # starts as early as possible.
            xt = xpool.tile([P, G_TOTAL, C], BF, tag="x")
            nc.sync.dma_start(out=xt[:, 0:G0, :], in_=x[:, 0:G0, :])
            idt = xpool.tile([P, P], BF, tag="id")
            nc.sync.dma_start(out=idt[:], in_=ident[:, :])
            nc.scalar.dma_start(out=xt[:, G0:, :], in_=x[:, G0:, :])

            g_off = 0
            for it, Gi in enumerate(G_ITERS):
                xg = xt[:, g_off : g_off + Gi, :]

                # PE stages the chunk into PSUM (xp = I.T @ xg, exact) for
                # DVE's broadcast operands, keeping DVE off the
                # GPSIMD-shared SBUF port.
                xp = ppool.tile([P, GMAX, C], FP, tag="xp")
                for c0 in range(0, Gi, MM_CHUNK):
                    c1 = min(c0 + MM_CHUNK, Gi)
                    nc.tensor.matmul(
                        out=xp[:, c0:c1, :], lhsT=idt[:], rhs=xg[:, c0:c1, :]
                    )

                otv = vpool.tile([P, Gi, R0], BF, tag="otv")
                otg = gpool.tile([P, Gi, K - R0], BF, tag="otg")

                # GPSIMD tail blocks (all-SBUF operands; DVE never touches
                # the shared pair).
                for i in range(I0, C - 1):
                    w = C - 1 - i
                    a = xg[:, :, i : i + 1].broadcast_to([P, Gi, w])
                    b = xg[:, :, i + 1 : C]
                    o = otg[:, :, _row[i] - R0 : _row[i] - R0 + w]
                    nc.gpsimd.tensor_mul(o, a, b)
                nc.scalar.dma_start(out=yg[:, g_off : g_off + Gi, :], in_=otg[:])

                for i in range(0, I0):
                    w = C - 1 - i
                    a = xp[:, 0:Gi, i : i + 1].broadcast_to([P, Gi, w])
                    b = xg[:, :, i + 1 : C]
                    nc.vector.tensor_mul(otv[:, :, _row[i] : _row[i] + w], a, b)
                nc.sync.dma_start(out=yv[:, g_off : g_off + Gi, :], in_=otv[:])

                g_off += Gi

    nc.finalize()
    return nc


def make_in_maps(x: np.ndarray) -> list[dict[str, np.ndarray]]:
    x = np.ascontiguousarray(x, dtype=np.float32).astype(NP_BF)
    eye = np.eye(P, dtype=np.float32).astype(NP_BF)
    return [
        {"x": x[c * BP : (c + 1) * BP].reshape(P, G_TOTAL, C), "ident": eye}
        for c in range(N_CORES)
    ]


def kernel(**inputs: np.ndarray) -> np.ndarray:
    global _nc_cache
    if _nc_cache is None:
        _nc_cache = build_bass()
    res = run_bass_kernel_spmd(
        _nc_cache, make_in_maps(inputs["inputs"]), list(range(N_CORES))
    ).results
    out = np.empty((B, H, W, K), dtype=np.float32)
    for c in range(N_CORES):
        sl = out[c * BP : (c + 1) * BP].reshape(P, G_TOTAL, K)
        sl[:, :, 0:R0] = res[c]["yv"].astype(np.float32)
        sl[:, :, R0:K] = res[c]["yg"].astype(np.float32)
    return out


# revision 8
# speedup vs baseline: 1.7428x; 1.1101x over previous
"""Triangular pairwise channel product on 8 Trainium2 NeuronCores.

out[b,h,w,k] = x[b,h,w,i_k] * x[b,h,w,j_k]  for the C*(C-1)/2 pairs
(i<j) in row-major (np.triu_indices) order.

Sharding: pure data parallel over batch — core c takes x[2c:2c+2].
Per core the 2*64*64 = 8192 spatial positions map to 128 SBUF
partitions (b_loc*64+h) x 64 groups (w).  Block i of the output (pairs
(i, i+1..63)) is one tensor_tensor multiply per group-chunk whose first
operand is x[..., i] broadcast via a step-0 access pattern.

Tracing showed the fp32 kernel is bound by DEVICE HBM bandwidth: all 8
cores stream stores simultaneously and sustain only ~350 GB/s each
(~2.8 TB/s device), so 528 MB of fp32 output floors at ~196 us no
matter the schedule.  The rel-err budget is 2e-2; bf16 products carry
~3e-3.  So x is cast to bf16 on the host, all products are computed
from bf16 inputs and stored as bf16 (264 MB device-wide, ~95 us
floor at the b16 DMA derate), and the host upcasts the result.

Compute paces the kernel now; design points from traces:

* Work split: blocks i < I0 on DVE (1 cyc/elem @0.96 + ~145 cyc/instr),
  tail blocks on GPSIMD (~1.9 ns/elem + ~190 ns/instr, otherwise idle).
* TT on DVE reads its second operand through the SBUF port pair GPSIMD
  uses — running both engines naively serializes them to ~2.3 cyc/elem.
  Fix: DVE's broadcast operand lives in PSUM (separate DVE read port).
  The PE — otherwise dead — stages each x chunk SBUF->PSUM via an
  identity matmul (bf16 in, exact fp32 out).
* Every store is single-engine: each engine owns separate DRAM output
  tensors (host concatenates channels during unshard).  Shared tiles
  made stores wait on both engines' sparse completion semaphores
  (up to ~17 us of rounding).  GPSIMD stores dispatch within ~30 ns of
  data-ready on their own ACT ring; DVE's semaphore thresholds still
  round into the next iteration, so the DVE output is split in two
  half stores (blocks 0..MID-1, MID..I0-1) — the first half is ready
  mid-iteration, pulling its dispatch a half-iteration earlier and
  smoothing the store stream that otherwise back-loads ~6 MB into a
  post-compute drain.
"""

import numpy as np

import concourse.bacc as bacc
import concourse.bass as bass
import concourse.mybir as mybir
import concourse.tile as tile
from concourse.bass_utils import run_bass_kernel_spmd

B, H, W, C = 16, 64, 64, 64
K = C * (C - 1) // 2  # 2016
N_CORES = 8
BP = B // N_CORES  # batch rows per core
P = BP * H         # 128 SBUF partitions
G_TOTAL = W        # position groups per partition
G_ITERS = [2, 6, 16, 16, 16, 6, 2]
assert sum(G_ITERS) == W
G0 = G_ITERS[0]
GMAX = max(G_ITERS)
# Blocks i >= I0 (widths 63-I0 .. 1) run on GPSIMD, the rest on DVE.
I0 = 27
MID = 14  # DVE half-store boundary
FP = mybir.dt.float32
BF = mybir.dt.bfloat16
NP_BF = mybir.dt.np(BF)
MM_CHUNK = 512 // C  # groups per identity-matmul (moving free dim <= 512)

_row = [0]
for _i in range(C):
    _row.append(_row[-1] + C - 1 - _i)
R1 = _row[MID]  # 791
R0 = _row[I0]   # 1350

_nc_cache = None


def build_bass() -> bass.Bass:
    # Bacc (not plain Bass): its compile() pipeline runs
    # generate_event_semaphores, which splits multi-wait instructions to
    # satisfy the TRN2 1-wait-per-instruction codegen limit.
    nc = bacc.Bacc(
        "TRN2",
        target_bir_lowering=False,
        debug=False,
        num_devices=N_CORES,
    )
    x = nc.dram_tensor("x", [P, G_TOTAL, C], BF, kind="ExternalInput")
    ident = nc.dram_tensor("ident", [P, P], BF, kind="ExternalInput")
    yva = nc.dram_tensor("yva", [P, G_TOTAL, R1], BF, kind="ExternalOutput")
    yvb = nc.dram_tensor("yvb", [P, G_TOTAL, R0 - R1], BF, kind="ExternalOutput")
    yg = nc.dram_tensor("yg", [P, G_TOTAL, K - R0], BF, kind="ExternalOutput")

    with tile.TileContext(nc) as tc:
        with (
            tc.tile_pool(name="xin", bufs=1) as xpool,
            tc.tile_pool(name="outva", bufs=3) as vapool,
            tc.tile_pool(name="outvb", bufs=3) as vbpool,
            tc.tile_pool(name="outg", bufs=3) as gpool,
            tc.tile_pool(name="xps", bufs=3, space="PSUM") as ppool,
        ):
            # One x tile; iteration 0's chunk loads on the SP ring, the
            # rest (+ identity) on the ACT ring so the first compute
            # starts as early as possible.
            xt = xpool.tile([P, G_TOTAL, C], BF, tag="x")
            nc.sync.dma_start(out=xt[:, 0:G0, :], in_=x[:, 0:G0, :])
            idt = xpool.tile([P, P], BF, tag="id")
            nc.sync.dma_start(out=idt[:], in_=ident[:, :])
            nc.scalar.dma_start(out=xt[:, G0:, :], in_=x[:, G0:, :])

            g_off = 0
            for it, Gi in enumerate(G_ITERS):
                xg = xt[:, g_off : g_off + Gi, :]

                # PE stages the chunk into PSUM (xp = I.T @ xg, exact) for
                # DVE's broadcast operands, keeping DVE off the
                # GPSIMD-shared SBUF port.
                xp = ppool.tile([P, GMAX, C], FP, tag="xp")
                for c0 in range(0, Gi, MM_CHUNK):
                    c1 = min(c0 + MM_CHUNK, Gi)
                    nc.tensor.matmul(
                        out=xp[:, c0:c1, :], lhsT=idt[:], rhs=xg[:, c0:c1, :]
                    )

                otva = vapool.tile([P, Gi, R1], BF, tag="otva")
                otvb = vbpool.tile([P, Gi, R0 - R1], BF, tag="otvb")
                otg = gpool.tile([P, Gi, K - R0], BF, tag="otg")

                # GPSIMD tail blocks (all-SBUF operands; DVE never touches
                # the shared pair), stored on the ACT ring nothing else
                # queues behind.
                for i in range(I0, C - 1):
                    w = C - 1 - i
                    a = xg[:, :, i : i + 1].broadcast_to([P, Gi, w])
                    b = xg[:, :, i + 1 : C]
                    o = otg[:, :, _row[i] - R0 : _row[i] - R0 + w]
                    nc.gpsimd.tensor_mul(o, a, b)
                nc.scalar.dma_start(out=yg[:, g_off : g_off + Gi, :], in_=otg[:])

                def dve_block(i, dst, base):
                    w = C - 1 - i
                    a = xp[:, 0:Gi, i : i + 1].broadcast_to([P, Gi, w])
                    b = xg[:, :, i + 1 : C]
                    nc.vector.tensor_mul(
                        dst[:, :, _row[i] - base : _row[i] - base + w], a, b
                    )

                for i in range(0, MID):
                    dve_block(i, otva, 0)
                nc.sync.dma_start(out=yva[:, g_off : g_off + Gi, :], in_=otva[:])
                for i in range(MID, I0):
                    dve_block(i, otvb, R1)
                nc.sync.dma_start(out=yvb[:, g_off : g_off + Gi, :], in_=otvb[:])

                g_off += Gi

    nc.finalize()
    return nc


def make_in_maps(x: np.ndarray) -> list[dict[str, np.ndarray]]:
    x = np.ascontiguousarray(x, dtype=np.float32).astype(NP_BF)
    eye = np.eye(P, dtype=np.float32).astype(NP_BF)
    return [
        {"x": x[c * BP : (c + 1) * BP].reshape(P, G_TOTAL, C), "ident": eye}
        for c in range(N_CORES)
    ]


def kernel(**inputs: np.ndarray) -> np.ndarray:
    global _nc_cache
    if _nc_cache is None:
        _nc_cache = build_bass()
    res = run_bass_kernel_spmd(
        _nc_cache, make_in_maps(inputs["inputs"]), list(range(N_CORES))
    ).results
    out = np.empty((B, H, W, K), dtype=np.float32)
    for c in range(N_CORES):
        sl = out[c * BP : (c + 1) * BP].reshape(P, G_TOTAL, K)
        sl[:, :, 0:R1] = res[c]["yva"].astype(np.float32)
        sl[:, :, R1:R0] = res[c]["yvb"].astype(np.float32)
        sl[:, :, R0:K] = res[c]["yg"].astype(np.float32)
    return out
